# revision 57
# baseline (speedup 1.0000x reference)
"""Distributed RoPE multi-head attention on one TRN2 chip (8 NeuronCores).

kernel(**inputs) -> np.ndarray [2, 4096, 1024]; bf16 TensorE, f32 PSUM.

Structure per core (tensor-parallel over heads, 2 heads/core):
  k-projection first (short prefix), then attention ic-chunks with the
  q/v projections and the next batch's work interleaved by the Tile
  scheduler; softmax exp split between ScalarE (exact) and DVE
  (Schraudolph fast-exp writing bf16 bits via an int16 view).
"""


import math

import numpy as np

import concourse.bass as bass
import concourse.tile as tile
from concourse import bacc, mybir

F32 = mybir.dt.float32
BF16 = mybir.dt.bfloat16
I16 = mybir.dt.int16
EXP = mybir.ActivationFunctionType.Exp
IDENT = mybir.ActivationFunctionType.Identity

THETA = 500.0
SCALE = 1.0
B, L, DM = 2, 4096, 1024
H, DH = 16, 64
HALF = DH // 2  # 32
N_CORES = 8
PCHUNK = (B * L) // N_CORES

NJ = L // 128          # 32 key tiles per batch
NIC = L // 512         # 8 query chunks of 512 per batch
XC = 512               # xT s-chunk size for projection
NXC = L // XC

# Schraudolph fast-exp on DVE: bf16 bits of exp(0.125*s) as
# round(s*SCH_C1 + SCH_C2) written to an int16 view of the p tile.
DVE_COLS = 384
SCH_C1 = 0.125 * 128.0 / math.log(2.0)
SCH_C2 = 16243.7


def build(dbg_mode=False):
    nc = bacc.Bacc("TRN2", target_bir_lowering=False, debug=False,
                   num_devices=N_CORES)

    # All weights arrive pre-transposed from the host in the exact SBUF
    # layout, so every weight DMA is a dense per-partition copy (the
    # strided rearranged loads cost thousands of 128B descriptors on
    # the SWDGE path — ~15-20us before the first matmul).
    xT = nc.dram_tensor("xT", [128, B * L * 8], BF16, kind="ExternalInput").ap()
    w_qk = nc.dram_tensor("w_qk", [128, 2 * 8 * 128], BF16,
                          kind="ExternalInput").ap()
    b_qk = nc.dram_tensor("b_qk", [128, 2], F32, kind="ExternalInput").ap()
    w_v = nc.dram_tensor("w_v", [128, 8 * 128], BF16,
                         kind="ExternalInput").ap()
    b_v = nc.dram_tensor("b_v", [1, 128], BF16, kind="ExternalInput").ap()
    w_out = nc.dram_tensor("w_out", [128, 8 * DM], BF16,
                           kind="ExternalInput").ap()
    b_out = nc.dram_tensor("b_out", [128, 8], F32, kind="ExternalInput").ap()
    cos_t = nc.dram_tensor("cos_t", [128, L], BF16, kind="ExternalInput").ap()
    sin_t = nc.dram_tensor("sin_t", [128, L], BF16, kind="ExternalInput").ap()
    # transposed layout [DM, PCHUNK]: the out-projection keeps w_out as
    # the stationary operand (no A2A-gated LDWEIGHTS hoisted into the
    # PE queue); the host transposes back. Column c = b*512 + hf*256 +
    # icm*64 + q maps to full[b, 512*(4*hf+icm) + 64*core + q] (query
    # ownership is striped so each batch's A2A splits into two halves).
    out = nc.dram_tensor("out", [DM, PCHUNK], F32, kind="ExternalOutput").ap()

    with tile.TileContext(nc) as tc:
        _body(nc, tc, xT, w_qk, b_qk, w_v, b_v, w_out, b_out, cos_t, sin_t,
              out)
    nc.finalize()
    return nc


def _body(nc, tc, xT, w_qk, b_qk, w_v, b_v, w_out, b_out, cos_t, sin_t, out):
    mm = nc.tensor.matmul

    with tc.tile_pool(name="const", bufs=1) as const, \
         tc.tile_pool(name="qkv", bufs=1) as qkv_pool, \
         tc.tile_pool(name="dram", bufs=1, space="DRAM") as dram:

        # Four quarter-A2As per batch: quarter qt carries ics 2qt and
        # 2qt+1, chunk c = the 64-query stripe of each ic that core c
        # out-projects. [c, icm, hd, q] with [hd, q] innermost so the
        # per-(ic, h) oN scatter writes 8 contiguous 8KB runs. Small
        # quarters keep the only exposed collective (the last one)
        # short even when the fabric is slow (~10-60GB/s run-to-run).
        a2a_in = [[dram.tile([N_CORES, 2, 128, 64], BF16,
                             name=f"a2ai{b}{qt}") for qt in range(4)]
                  for b in range(B)]
        a2a_out = [[dram.tile([N_CORES, 2, 128, 64], BF16,
                              name=f"a2ao{b}{qt}") for qt in range(4)]
                   for b in range(B)]
        # warmup collective buffers (garbage data, absorbs the ~11.5us
        # first-collective CC-stream setup off the critical path)
        wu_in = dram.tile([N_CORES, 16], BF16, name="wui")
        wu_out = dram.tile([N_CORES, 16], BF16, name="wuo")

        wqk = const.tile([128, 16 * 128], BF16)
        wqk_r = wqk.rearrange("p (ct dmt c) -> p ct dmt c", ct=2, dmt=8)
        nc.gpsimd.dma_start(wqk[:], w_qk[:])

        # wv/bout are emitted later (after xt00) so the gpsimd ring
        # serves the first projection's inputs first.
        wv = const.tile([128, 8 * 128], BF16)
        wv_r = wv.rearrange("p (dmt c) -> p dmt c", dmt=8)

        bqk = const.tile([128, 2], F32)
        nc.sync.dma_start(bqk[:], b_qk[:])
        bv = const.tile([1, 128], BF16)
        nc.sync.dma_start(bv[:], b_v[:])
        bout = const.tile([128, 8], F32)

        wout = const.tile([128, 8 * DM], BF16)
        wout_r = wout.rearrange("p (ct n) -> p ct n", ct=8)

        ones_row = const.tile([1, 128], BF16)
        nc.vector.memset(ones_row[:], 1.0)

        # Warm the ScalarE exp table (~2.7us ACT_TABLE_LOAD) off the
        # critical path, before the first real softmax tile.
        warm = const.tile([1, 8], F32)
        nc.vector.memset(warm[:], 0.0)
        warm2 = const.tile([1, 8], F32)
        nc.scalar.activation(warm2[:], warm[:], EXP)

        # cos/sin ride the sync DMA queue, parallel with the gpsimd
        # queue that carries weights + xt chunks. Only the first two
        # chunks' worth up front; the rest is emitted after the first
        # rope so chunk 0's rotate DMAs aren't stuck behind 2MB.
        cos_sb = const.tile([128, L], BF16)
        sin_sb = const.tile([128, L], BF16)
        nc.sync.dma_start(cos_sb[:, 0:2 * XC], cos_t[:, 0:2 * XC])
        nc.sync.dma_start(sin_sb[:, 0:2 * XC], sin_t[:, 0:2 * XC])

        qT = [qkv_pool.tile([128, L], BF16, tag=f"qT{b}", name=f"qT{b}")
              for b in range(B)]
        kT = [qkv_pool.tile([128, L], BF16, tag=f"kT{b}", name=f"kT{b}")
              for b in range(B)]
        vaug = [qkv_pool.tile([128, NJ * 130], BF16, tag=f"va{b}",
                              name=f"va{b}") for b in range(B)]

        with tc.tile_pool(name="xt", bufs=8) as xt_pool, \
             tc.tile_pool(name="rope", bufs=2) as rope_pool, \
             tc.tile_pool(name="pp", bufs=2, space="PSUM") as proj_psum, \
             tc.tile_pool(name="sp", bufs=2, space="PSUM") as s_psum, \
             tc.tile_pool(name="op", bufs=2, space="PSUM") as o_psum, \
             tc.tile_pool(name="pt", bufs=4) as p_pool, \
             tc.tile_pool(name="nrm", bufs=2) as n_pool:

            # host layout: [128, B, NXC, 8, XC] — one contiguous 8KB run
            # per partition per (b, xc) chunk
            xT_d = xT.rearrange("p (b xc dmt s) -> p b xc dmt s",
                                b=B, xc=NXC, dmt=8)

            def load_xt(b, xc, eng=None):
                xt = xt_pool.tile([128, 8, XC], BF16)
                (eng or nc.gpsimd).dma_start(xt[:], xT_d[:, b, xc, :, :])
                return xt

            def rope_core(b, xc, qb, rot, nct):
                # qb/rot are [128, nct*XC] (q | k); one rotate DMA set
                # covers every ct — the ~650ns/DMA ring-serialization
                # cost is per trigger, not per byte.
                s0 = xc * XC
                w = nct * XC
                for h2 in range(4):
                    src = h2 * 32 + (32 if h2 % 2 == 0 else -32)
                    nc.sync.dma_start(rot[h2 * 32:(h2 + 1) * 32, 0:w],
                                      qb[src:src + 32, 0:w])
                tsin = rope_pool.tile([128, 2 * XC], BF16, tag="tsin")
                tcos = rope_pool.tile([128, 2 * XC], BF16, tag="tcos")
                for ct in range(nct):
                    c0 = ct * XC
                    nc.vector.tensor_mul(tsin[:, c0:c0 + XC],
                                         rot[:, c0:c0 + XC],
                                         sin_sb[:, s0:s0 + XC])
                    nc.vector.tensor_mul(tcos[:, c0:c0 + XC],
                                         qb[:, c0:c0 + XC],
                                         cos_sb[:, s0:s0 + XC])
                tgts = (qT[b], kT[b])
                for ct in range(nct):
                    c0 = ct * XC
                    nc.vector.tensor_add(tgts[ct][:, s0:s0 + XC],
                                         tcos[:, c0:c0 + XC],
                                         tsin[:, c0:c0 + XC])

            def proj_qk(b, ct, xc, xt):
                ps = proj_psum.tile([128, XC], F32, tag="ps", name="ps")
                for dmt in range(8):
                    mm(ps[:], wqk_r[:, ct, dmt, :], xt[:, dmt, :],
                       start=(dmt == 0), stop=(dmt == 7))
                s0 = xc * XC
                tgt = (qT[b], kT[b])[ct]
                qb = rope_pool.tile([128, 2 * XC], BF16, tag="qb")
                nc.scalar.activation(qb[:, 0:XC], ps[:], IDENT,
                                     bias=bqk[:, ct:ct + 1])
                rot = rope_pool.tile([128, 2 * XC], BF16, tag="rot")
                for h2 in range(4):
                    src = h2 * 32 + (32 if h2 % 2 == 0 else -32)
                    nc.sync.dma_start(rot[h2 * 32:(h2 + 1) * 32, 0:XC],
                                      qb[src:src + 32, 0:XC])
                tsin = rope_pool.tile([128, 2 * XC], BF16, tag="tsin")
                nc.vector.tensor_mul(tsin[:, 0:XC], rot[:, 0:XC],
                                     sin_sb[:, s0:s0 + XC])
                tcos = rope_pool.tile([128, 2 * XC], BF16, tag="tcos")
                nc.vector.tensor_mul(tcos[:, 0:XC], qb[:, 0:XC],
                                     cos_sb[:, s0:s0 + XC])
                nc.vector.tensor_add(tgt[:, s0:s0 + XC], tcos[:, 0:XC],
                                     tsin[:, 0:XC])

            def proj_qkf(b, xc, xt):
                # fused q+k projection + rope for one chunk (b0 path)
                qb = rope_pool.tile([128, 2 * XC], BF16, tag="qb")
                for ct in range(2):
                    ps = proj_psum.tile([128, XC], F32, tag="ps", name="ps")
                    for dmt in range(8):
                        mm(ps[:], wqk_r[:, ct, dmt, :], xt[:, dmt, :],
                           start=(dmt == 0), stop=(dmt == 7))
                    nc.scalar.activation(qb[:, ct * XC:(ct + 1) * XC],
                                         ps[:], IDENT,
                                         bias=bqk[:, ct:ct + 1])
                rot = rope_pool.tile([128, 2 * XC], BF16, tag="rot")
                rope_core(b, xc, qb, rot, 2)

            def proj_v(b, xc, xt):
                va4 = vaug[b].rearrange("p (st h c) -> p st h c", st=NJ, h=2)
                for u in range(XC // 128):
                    st = (xc * XC + u * 128) // 128
                    ps = proj_psum.tile([128, 128], F32, tag="ps", name="vps")
                    for dmt in range(8):
                        mm(ps[:], xt[:, dmt, u * 128:(u + 1) * 128],
                           wv_r[:, dmt, :], start=(dmt == 0), stop=False)
                    mm(ps[:], ones_row[:], bv[:], start=False, stop=True)
                    nc.vector.tensor_copy(
                        va4[:, st, :, 0:64],
                        ps[:].rearrange("p (h c) -> p h c", h=2))

            def emit_qv(b, xc):
                xt = load_xt(b, xc)
                proj_qk(b, 0, xc, xt)
                proj_v(b, xc, xt)

            def load_ot(b, hf, gate=None):
                # one out-proj half: [128 hd, 8 src-core, 256 (icm q)],
                # assembled from two quarter-A2A outputs. Rides the
                # scalar DMA ring so the big loads never block the sync
                # ring; the gate (a tiny read of the last attention
                # drain) pins the out-proj into the tail — without it
                # the scheduler hoists the matmuls into the attention
                # phase and the in-order PE queue stalls.
                ot = p_pool.tile([128, 8 * 256], BF16, tag="ot", name="ot",
                                 bufs=2)
                if gate is not None:
                    nc.scalar.dma_start(ot[0:1, 0:1], gate[0:1, 0:1])
                d4 = ot.rearrange("p (ct icm q) -> p ct icm q", ct=8, icm=4)
                for icm in range(4):
                    nc.scalar.dma_start(
                        d4[:, :, icm, :],
                        a2a_out[b][2 * hf + icm // 2][:, icm % 2]
                        .rearrange("j p q -> p j q"))
                return ot.rearrange("p (ct s) -> p ct s", ct=8)

            def outproj_pair(otr, mp, col0, ring):
                # two out^T row-blocks per store trigger (the ~650ns
                # ring-serialization cost is per trigger); stores
                # alternate between the sync and scalar rings.
                ob = n_pool.tile([128, 512], F32, tag="ob", name="ob",
                                 bufs=4)
                for half in range(2):
                    m = 2 * mp + half
                    ps = proj_psum.tile([128, 512], F32, tag="ps",
                                        name="fps")
                    for ct in range(8):
                        mm(ps[:, 0:256],
                           wout_r[:, ct, m * 128:(m + 1) * 128],
                           otr[:, ct, :],
                           start=(ct == 0), stop=(ct == 7))
                    nc.vector.tensor_scalar_add(
                        ob[:, half * 256:(half + 1) * 256], ps[:, 0:256],
                        bout[:, m:m + 1])
                dst = out[2 * mp * 128:(2 * mp + 2) * 128, col0:col0 + 256]
                ring.dma_start(
                    dst.rearrange("(blk p) c -> p blk c", blk=2),
                    ob[:].rearrange("p (blk c) -> p blk c", blk=2))

            def emit_proj_k(b, preloaded=None):
                va4 = vaug[b].rearrange("p (st h c) -> p st h c", st=NJ, h=2)
                nc.vector.memset(va4[:, :, :, 64:65], 1.0)
                for xc in range(NXC):
                    xt = preloaded[xc] if preloaded else load_xt(b, xc)
                    proj_qk(b, 1, xc, xt)

            # Each ic's normalize finishers run at jj==2/3 of the NEXT
            # ic's j-loop; the half-A2A whose last input they store is
            # emitted at jj==4 (after them, so the collective sees the
            # write).
            pending = [None]
            last_od = []

            def emit_attn(b, ic, dve_cols, pre_j=None, at4=None,
                          od_sink=None):
                prev = pending[0]

                def pj(jj):
                    if prev is not None:
                        if jj == 3:
                            prev[0]()
                        elif jj == 4:
                            prev[1]()
                    if jj == 5 and at4 is not None:
                        at4()
                    if pre_j is not None:
                        pre_j(jj)
                pending[0] = _attention_ic(
                    nc, b, ic, qT[b], kT[b], vaug[b], s_psum, o_psum,
                    p_pool, n_pool, proj_psum, a2a_in, dve_cols, ones_row,
                    pj, od_sink)

            def emit_a2a(b, qt):
                nc.gpsimd.collective_compute(
                    "AllToAll", mybir.AluOpType.bypass,
                    replica_groups=[list(range(N_CORES))],
                    ins=[a2a_in[b][qt].opt()], outs=[a2a_out[b][qt].opt()])

            # ---- batch 0: the whole projection is fused into attention
            # chunk ic0, one x-chunk (q+k+v) per 4 key-tiles, one block
            # ahead of the S prefetch that consumes it. First softmax
            # lands ~10us in; PE never idles waiting for the k-pass. ----
            va4_0 = vaug[0].rearrange("p (st h c) -> p st h c", st=NJ, h=2)
            nc.vector.memset(va4_0[:, :, :, 64:65], 1.0)
            xt00 = load_xt(0, 0)
            xt_b0 = {0: xt00, 1: load_xt(0, 1, nc.sync)}
            nc.gpsimd.dma_start(wv[:], w_v[:])
            nc.gpsimd.dma_start(bout[:], b_out[:])
            proj_qkf(0, 0, xt00)

            def pre_j0(jj):
                # v(0) delayed past the first S-pair so the first exp
                # fires as early as possible. x chunks prefetch 2 slots
                # ahead, alternating gpsimd/sync rings — the startup
                # inflow is near the HBM roofline, so both rings carry x.
                if jj == 1:
                    proj_v(0, 0, xt00)
                if jj % 4 == 0:
                    xc_l = jj // 4 + 2
                    if xc_l < NXC:
                        s0 = xc_l * XC
                        nc.gpsimd.dma_start(cos_sb[:, s0:s0 + XC],
                                            cos_t[:, s0:s0 + XC])
                        nc.gpsimd.dma_start(sin_sb[:, s0:s0 + XC],
                                            sin_t[:, s0:s0 + XC])
                        xt_b0[xc_l] = load_xt(
                            0, xc_l, nc.gpsimd if xc_l % 2 == 0 else nc.sync)
                    xc_p = jj // 4 + 1
                    if xc_p < NXC:
                        proj_qkf(0, xc_p, xt_b0[xc_p])
                        proj_v(0, xc_p, xt_b0[xc_p])
                if jj == 28:
                    nc.gpsimd.dma_start(wout[:], w_out[:])
                if jj == 24:
                    # warmup collective (garbage payload): pays the
                    # first-collective CC setup here, where nothing on
                    # the gpsimd ring is urgent for ~70us
                    nc.gpsimd.collective_compute(
                        "AllToAll", mybir.AluOpType.bypass,
                        replica_groups=[list(range(N_CORES))],
                        ins=[wu_in.opt()], outs=[wu_out.opt()])

            # batch-1 projection work is interleaved INTO attn(b0)'s
            # emission so its PE-queue slots sit inside the b0 phase:
            # k-chunks during ics 2-5 (their xt tiles are KEPT — the
            # 8-buf xt pool holds exactly chunks 0-7 — so the q/v
            # projections reuse them with no reload), q+v of chunk 0
            # during ic6.
            va4_1 = vaug[1].rearrange("p (st h c) -> p st h c", st=NJ, h=2)
            nc.vector.memset(va4_1[:, :, :, 64:65], 1.0)
            xt_b1 = {}

            def mk_pre_j(ic):
                if ic == 0:
                    return pre_j0
                if 2 <= ic <= 5:
                    def hook(jj, _ic=ic):
                        if jj in (0, 16):
                            xc = 2 * (_ic - 2) + (0 if jj == 0 else 1)
                            xt_b1[xc] = load_xt(1, xc)
                            proj_qk(1, 1, xc, xt_b1[xc])
                    return hook
                if ic == 6:
                    def hook(jj):
                        if jj == 0:
                            proj_qk(1, 0, 0, xt_b1[0])
                            proj_v(1, 0, xt_b1[0])
                    return hook
                return None

            for ic in range(NIC):
                at4 = None
                if ic in (2, 4, 6):
                    at4 = (lambda k: lambda: emit_a2a(0, k))(ic // 2 - 1)
                emit_attn(0, ic, DVE_COLS, mk_pre_j(ic), at4=at4)

            # q/v of batch 1 fused into attn(b1, ic0): that phase is
            # ScalarE-bound, so the projection rides in PE slack.
            def pre_j1(jj):
                if jj % 4 == 0 and jj // 4 + 1 < NXC:
                    xc = jj // 4 + 1
                    xt = xt_b1[xc]
                    proj_qk(1, 0, xc, xt)
                    proj_v(1, xc, xt)

            for ic in range(NIC):
                at4 = None
                if ic == 0:
                    at4 = lambda: emit_a2a(0, 3)
                elif ic in (2, 4, 6):
                    at4 = (lambda k: lambda: emit_a2a(1, k))(ic // 2 - 1)
                emit_attn(1, ic, DVE_COLS, pre_j1 if ic == 0 else None,
                          at4=at4, od_sink=last_od if ic == 7 else None)
            fin = pending[0]
            fin[0]()
            fin[1]()
            emit_a2a(1, 3)

            # tail: every out-proj half except b1-hf1 is gated on the
            # LAST attention drain (od of b1-ic7-h0) so it fills the
            # final norm chain + A2A(b1, hf1) window, where the PE is
            # otherwise idle; b1-hf1 follows when its collective lands.
            for b_, hf_, col0 in ((0, 0, 0), (0, 1, 256), (1, 0, 512),
                                  (1, 1, 768)):
                ot = load_ot(b_, hf_,
                             gate=last_od[0] if (b_, hf_) != (1, 1) else None)
                for mp in range(4):
                    ring = nc.sync if mp % 2 == 0 else nc.scalar
                    outproj_pair(ot, mp, col0, ring)


def _attention_ic(nc, b, ic, qTb, kTb, vaugb, s_psum, o_psum, p_pool, n_pool,
                  proj_psum, a2a_in, dve_cols, ones_row, pre_j=None,
                  od_sink=None):
    mm = nc.tensor.matmul
    va = vaugb

    def mm1(j, s_ps):
        for h in range(2):
            p0 = 64 * h
            mm(s_ps[:, h * 512:(h + 1) * 512],
               kTb[p0:p0 + 64, j * 128:(j + 1) * 128],
               qTb[p0:p0 + 64, ic * 512:(ic + 1) * 512],
               start=True, stop=True, tile_position=(p0, 0))

    o_ps = [o_psum.tile([65, 512], F32, tag="o", name=f"o{h}")
            for h in range(2)]
    # S runs two steps ahead of its exp and is emitted BEFORE the PVs:
    # the PSUM-slot WAR dep makes S(j+2) start right after exp(j)
    # drains its bank, so ScalarE is never gated on PV progress.
    s_tiles = {}
    for j0 in range(2):
        if pre_j is not None:
            pre_j(j0)
        s_tiles[j0] = s_psum.tile([128, 1024], F32, name="s_t")
        mm1(j0, s_tiles[j0])
    for j in range(NJ):
        if j + 2 < NJ:
            if pre_j is not None:
                pre_j(j + 2)
            s_tiles[j + 2] = s_psum.tile([128, 1024], F32, name="s_t")
            mm1(j + 2, s_tiles[j + 2])
        # per-engine p tiles split at the PV boundary: ScalarE (exact
        # exp, head 0) and DVE (Schraudolph, head 1) write different
        # tiles, so no cross-engine ordering serializes the exp chain
        # and the s-bank recycles at max(ACT, TS) instead of ACT+TS.
        p_s = p_pool.tile([128, 512], BF16, tag="p_s")
        p_d = p_pool.tile([128, 512], BF16, tag="p_d")
        nc.scalar.activation(p_s[:], s_tiles[j][:, 0:512],
                             EXP, scale=0.125)
        if dve_cols:
            nc.vector.tensor_scalar(
                p_d[:].bitcast(I16), s_tiles[j][:, 512:1024],
                SCH_C1, SCH_C2,
                mybir.AluOpType.mult, mybir.AluOpType.add)
        else:
            nc.scalar.activation(p_d[:], s_tiles[j][:, 512:1024],
                                 EXP, scale=0.125)
        for h, p_h in enumerate((p_s, p_d)):
            mm(o_ps[h][:],
               va[:, j * 130 + h * 65: j * 130 + (h + 1) * 65],
               p_h[:],
               start=(j == 0), stop=(j == NJ - 1))
        del s_tiles[j]

    # Stage A (here): drain o+rowsum off PSUM, reciprocal of the rowsum.
    # Stage B (returned closures, run inside the NEXT ic's j-loop, so
    # the in-order PE queue never stalls on the reciprocal and the
    # half-A2A can be emitted after its last input store): broadcast
    # the reciprocal across 64 partitions with a rank-1 matmul into
    # PSUM, normalize, scatter-store into the half-A2A buffer.
    finishers = []
    for h in range(2):
        od = n_pool.tile([128, 512], F32, tag="od")
        # drain on ScalarE (identity): keeps the DVE free for the
        # steady-state Schraudolph half
        nc.scalar.activation(od[0:65, :], o_ps[h][0:65, :], IDENT)
        rs0 = n_pool.tile([1, 512], F32, tag="rs0")
        nc.sync.dma_start(rs0[:], od[64:65, :])
        rcp32 = n_pool.tile([1, 512], F32, tag="rcp32")
        nc.vector.reciprocal_approx_fast(rcp32[:], rs0[:])
        rcp = n_pool.tile([1, 512], BF16, tag="rcp")
        nc.vector.tensor_copy(rcp[:], rcp32[:])
        if od_sink is not None and h == 0:
            od_sink.append(rcp)

        def fin(h=h, od=od, rcp=rcp):
            bcp = proj_psum.tile([128, 512], F32, tag="ps", name="bcp")
            mm(bcp[0:64, :], ones_row[:, 0:64], rcp[:],
               start=True, stop=True)
            oN = n_pool.tile([64, 512], BF16, tag="oN")
            nc.vector.tensor_mul(oN[:], od[0:64, :], bcp[0:64, :])
            dst = a2a_in[b][ic // 2][:, ic % 2, 64 * h:64 * (h + 1), :]
            nc.sync.dma_start(dst.rearrange("c p q -> p c q"),
                              oN[:].rearrange("p (c q) -> p c q", c=8))
        finishers.append(fin)
    return finishers


def make_tables():
    f = np.arange(HALF, dtype=np.float64)
    freqs = THETA ** (-f / HALF)
    ang = SCALE * np.outer(np.arange(L, dtype=np.float64), freqs)
    c32 = np.cos(ang.T).astype(np.float32)
    s32 = np.sin(ang.T).astype(np.float32)
    cos128 = np.concatenate([c32, c32, c32, c32], axis=0)
    sin128 = np.concatenate([-s32, s32, -s32, s32], axis=0)
    return np.ascontiguousarray(cos128), np.ascontiguousarray(sin128)


def make_in_maps(x, w_qkv, b_qkv, w_out, b_out):
    import ml_dtypes
    BF = ml_dtypes.bfloat16
    x = np.asarray(x, dtype=np.float32)
    w_qkv = np.asarray(w_qkv, dtype=np.float32)
    b_qkv = np.asarray(b_qkv, dtype=np.float32)
    # dense SBUF layouts: [p, ct, dmt, c] etc. (p = dm-row within the
    # 128-row dmt chunk), so on-device weight DMAs are contiguous.
    w_out_t = np.ascontiguousarray(
        np.asarray(w_out, dtype=np.float32).reshape(8, 128, DM)
        .transpose(1, 0, 2).reshape(128, 8 * DM)).astype(BF)
    b_out_t = np.ascontiguousarray(
        np.asarray(b_out, dtype=np.float32).reshape(8, 128).T)
    xT = np.ascontiguousarray(
        x.reshape(B, NXC, XC, 8, 128).transpose(4, 0, 1, 3, 2).reshape(
            128, -1)).astype(BF)
    cos128, sin128 = make_tables()
    cos128 = cos128.astype(BF)
    sin128 = sin128.astype(BF)
    in_maps = []
    for i in range(N_CORES):
        h0, h1 = 2 * i, 2 * i + 1

        def wslice(base):
            return [w_qkv[:, base + 64 * h0: base + 64 * h0 + 64],
                    w_qkv[:, base + 64 * h1: base + 64 * h1 + 64]]

        def bslice(base):
            return [b_qkv[base + 64 * h0: base + 64 * h0 + 64],
                    b_qkv[base + 64 * h1: base + 64 * h1 + 64]]

        # [1024 dm, 256] -> [p, ct, dmt, c] dense
        w_qk = np.concatenate(wslice(0) + wslice(DM), axis=1)  # [1024, 256]
        w_qk = np.ascontiguousarray(
            w_qk.reshape(8, 128, 2, 128).transpose(1, 2, 0, 3)
            .reshape(128, 2048)).astype(BF)
        b_qk = np.ascontiguousarray(
            np.stack([np.concatenate(bslice(0)),
                      np.concatenate(bslice(DM))], axis=1),
            dtype=np.float32)  # [128, 2]
        w_v = np.concatenate(wslice(2 * DM), axis=1)  # [1024, 128]
        w_v = np.ascontiguousarray(
            w_v.reshape(8, 128, 128).transpose(1, 0, 2)
            .reshape(128, 1024)).astype(BF)
        b_v = np.ascontiguousarray(
            np.concatenate(bslice(2 * DM)).reshape(1, 128),
            dtype=np.float32).astype(BF)
        in_maps.append({
            "xT": xT, "w_qk": w_qk, "b_qk": b_qk, "w_v": w_v, "b_v": b_v,
            "w_out": w_out_t, "b_out": b_out_t,
            "cos_t": cos128, "sin_t": sin128,
        })
    return in_maps


def gather_out(results):
    # out col c = b*512 + hf*256 + icm*64 + q on core i holds
    # full[b, 512*(4*hf+icm) + 64*i + q, :].
    full = np.zeros((B, L, DM), dtype=np.float32)
    for i in range(N_CORES):
        o = results[i]["out"]  # [DM, 1024]
        for b in range(B):
            for ic in range(8):
                src = o[:, b * 512 + ic * 64: b * 512 + ic * 64 + 64]
                full[b, 512 * ic + 64 * i: 512 * ic + 64 * i + 64, :] = src.T
    return full


# ---------------- harness entry ----------------

_NC_CACHE = {}


def _run(x, w_qkv, b_qkv, w_out, b_out, trace=False):
    from concourse.bass_utils import run_bass_kernel_spmd

    if "nc" not in _NC_CACHE:
        _NC_CACHE["nc"] = build()
    nc = _NC_CACHE["nc"]
    in_maps = make_in_maps(x, w_qkv, b_qkv, w_out, b_out)
    res = run_bass_kernel_spmd(nc, in_maps, list(range(N_CORES)), trace=trace)
    return gather_out(res.results), res


def kernel(x, w_qkv, b_qkv, w_out, b_out):
    full, _ = _run(x, w_qkv, b_qkv, w_out, b_out, trace=False)
    return full

